# revision 1
# baseline (speedup 1.0000x reference)
"""GAT edge-softmax kernel for 8 TRN2 NeuronCores (Bass/Tile).

Reference (per edge e, destination row[e], source col[e], H=8 heads):
    e_eh  = leakyrelu(aa[h,:F] @ x[row[e]] + aa[h,F:] @ x[col[e]], 0.2)
    out   = segment_softmax(e, grouped by row[e])          -> [H, E]

Distribution / algorithm:
  * Edges are grouped by destination node (the segment key).  Nodes are
    sorted by in-degree and 128-node tiles are dealt round-robin to the
    8 cores, so all cores run one identical (SPMD) padded schedule.
  * The score splits as s_row[row] + s_col[col].  s_row is a tiny
    per-node matmul on device.  For s_col, the host ships the gathered
    x-halo x[col[e]] (per the sharding hint), packed 4 edge-slots per
    128-deep K column; the device computes all per-edge s_col scores
    with a single stationary block-diagonal weight matrix
    (4 x [32F -> 8H]) streamed on the PE -- S/4 columns total.
  * PE output [32, S/4] (fp32 PSUM) is cast to fp16 and moved into the
    [128 node-partitions, ...] softmax layout with the hardware DMA
    transpose (2-byte xbar).
  * Segment softmax is then a free-dim reduction per node row.  Padded
    slots carry x=0 => s_col=0, and their exp contribution
    exp(lrelu(s_row)) * npad is subtracted from the denominator in
    closed form (no masks).
  * exp() is taken without max-subtraction: scores are O(10), safely
    inside f32 exp range; the reference's eps=1e-12 is negligible in
    both formulations.
  * The host unpads/unpermutes the padded per-core outputs to [H, E].
"""

from contextlib import ExitStack

import numpy as np

P = 128          # SBUF partitions
H = 8            # attention heads
F = 32           # in_features
Q = 4            # edge slots packed per K=128 matmul column
ALPHA = 0.2      # LeakyReLU slope
EPS = 1e-12
NCORES = 8
MM_N = 512       # matmul free-dim chunk (one PSUM bank)
BATCH_C = 48     # target c-groups (x128 cols) per pipeline batch

_prog_cache: dict = {}
LAST_RESULT = None  # BassKernelResults of the most recent kernel() call


# --------------------------------------------------------------------------
# host-side sharding / layout prep
# --------------------------------------------------------------------------

def _host_prep(x, aa, row, col, ncores=NCORES):
    N, Fdim = x.shape
    E = row.shape[0]
    assert Fdim == F and aa.shape == (H, 2 * F)
    row = np.asarray(row, dtype=np.int64)
    col = np.asarray(col, dtype=np.int64)
    x = np.asarray(x, np.float32)
    aa = np.asarray(aa, np.float32)

    deg = np.bincount(row, minlength=N)
    order = np.argsort(-deg, kind="stable")          # node ids, degree desc
    G = -(-N // P)                                   # global 128-node tiles
    G = -(-G // ncores) * ncores                     # multiple of ncores
    NG = G * P
    order_pad = np.concatenate([order, np.zeros(NG - N, np.int64)])
    ghost = np.zeros(NG, bool)
    ghost[N:] = True
    rank = np.empty(N, np.int64)
    rank[order] = np.arange(N)

    deg_sorted = np.concatenate([deg[order], np.zeros(NG - N, np.int64)])
    Dt = deg_sorted.reshape(G, P).max(axis=1)
    J = G // ncores
    Dj = Dt.reshape(J, ncores).max(axis=1).astype(np.int64)   # stripe max
    Dj = ((Dj + Q - 1) // Q) * Q                     # multiple of Q
    Cj = Dj // Q                                     # c-groups per tile
    C0 = np.concatenate([[0], np.cumsum(Cj)]).astype(np.int64)
    C_tot = int(C0[-1])
    SD = C_tot * Q                                   # padded slots per node

    # group edges by destination rank
    er = rank[row]
    sidx = np.argsort(er, kind="stable")
    er_s = er[sidx]
    start = np.searchsorted(er_s, np.arange(NG + 1))
    k = np.arange(E) - start[er_s]                   # slot within segment
    p_e = er_s % P
    tg_e = er_s // P
    j_e = tg_e // ncores
    c_e = (tg_e % ncores).astype(np.int32)
    cg_e = C0[j_e] + k // Q                          # global c-group
    q_e = k % Q

    # xg4: [ncores][128, C_tot*128] fp16, row 32q+f, col c*128+p
    col_slot = np.full((ncores, C_tot, P, Q), -1, np.int64)
    col_slot[c_e, cg_e, p_e, q_e] = col[sidx]
    xg4 = np.empty((ncores, P, C_tot * P), np.float16)
    for cc in range(ncores):
        cs = col_slot[cc]
        xs = x[np.clip(cs, 0, None)]                 # [C,128,Q,F]
        xs[cs < 0] = 0.0
        xg4[cc] = np.ascontiguousarray(
            xs.transpose(2, 3, 0, 1).reshape(P, C_tot * P)).astype(np.float16)

    # per-core own-node features (transposed) and pad counts
    xto = np.zeros((ncores, F, J * P), np.float32)
    npad = np.empty((ncores, P, J), np.float32)
    own_deg = deg_sorted.reshape(G, P)               # [G, P]
    for cc in range(ncores):
        gt = np.arange(J) * ncores + cc              # global tiles of core
        nodes = order_pad.reshape(G, P)[gt]          # [J, P]
        gmask = ghost.reshape(G, P)[gt]
        xv = x[nodes]                                # [J, P, F]
        xv[gmask] = 0.0
        xto[cc] = xv.reshape(J * P, F).T
        npad[cc] = (Dj[:, None] - own_deg[gt]).T.astype(np.float32)

    aarT = np.ascontiguousarray(aa[:, :F].T)         # [32, 8] f32
    aablk = np.zeros((P, Q * H), np.float16)         # [128, 32]
    for q in range(Q):
        aablk[q * F:(q + 1) * F, q * H:(q + 1) * H] = aa[:, F:].T
    # batches of tiles for the device pipeline
    batches = []
    j0 = 0
    while j0 < J:
        j1 = j0
        cw = 0
        while j1 < J and (cw == 0 or cw + Cj[j1] <= BATCH_C):
            cw += Cj[j1]
            j1 += 1
        if cw > 0:
            batches.append((int(j0), int(j1)))
        if j1 == j0:
            j1 += 1
        j0 = j1

    out_flat = (p_e * (32 * C_tot) + cg_e * 32 + q_e * H).astype(np.int64)

    meta = dict(G=G, J=J, C_tot=C_tot, ncores=ncores,
                Cj=tuple(int(c) for c in Cj), batches=tuple(batches),
                sidx=sidx, c_e=c_e, out_flat=out_flat, E=E)
    return dict(xg4=xg4, xto=xto, aarT=aarT, aablk=aablk, npad=npad), meta


# --------------------------------------------------------------------------
# device program
# --------------------------------------------------------------------------

def _build_program(J, C_tot, ncores, Cj, batches, debug=False):
    import concourse.bacc as bacc
    import concourse.tile as tile
    from concourse import mybir

    f32 = mybir.dt.float32
    f16 = mybir.dt.float16
    Cj = list(Cj)
    C0 = [0]
    for c in Cj:
        C0.append(C0[-1] + c)

    nc = bacc.Bacc("TRN2", target_bir_lowering=False, debug=False,
                   num_devices=ncores)

    xg_d = nc.dram_tensor("xg4", [P, C_tot * P], f16, kind="ExternalInput")
    xto_d = nc.dram_tensor("xto", [F, J * P], f32, kind="ExternalInput")
    aar_d = nc.dram_tensor("aar", [F, H], f32, kind="ExternalInput")
    ablk_d = nc.dram_tensor("ablk", [P, Q * H], f16, kind="ExternalInput")
    npad_d = nc.dram_tensor("npad", [P, J], f32, kind="ExternalInput")
    out_d = nc.dram_tensor("out", [P, 32 * C_tot], f32, kind="ExternalOutput")
    # DRAM bounce for the score transpose: per batch a contiguous
    # [32*W, 128] block (xbar ucode transposes [M, 128] -> [128, M]).
    nbatch = len(batches)
    s16d = nc.dram_tensor("s16d", [nbatch, 32 * BATCH_C * P], f16)
    if debug:
        dbg_s16 = nc.dram_tensor("dbg_s16", [32, BATCH_C * P], f16,
                                 kind="ExternalOutput")
        dbg_e16 = nc.dram_tensor("dbg_e16", [P, 32 * BATCH_C], f16,
                                 kind="ExternalOutput")
        dbg_eb = nc.dram_tensor("dbg_eb", [P, 32 * BATCH_C], f32,
                                kind="ExternalOutput")
        dbg_srow = nc.dram_tensor("dbg_srow", [P, J * H], f32,
                                  kind="ExternalOutput")

    with tile.TileContext(nc) as tc, ExitStack() as ctx:
        const = ctx.enter_context(tc.tile_pool(name="const", bufs=1))
        xtp = ctx.enter_context(tc.tile_pool(name="xt", bufs=2))
        psc = ctx.enter_context(tc.tile_pool(name="psc", bufs=4, space="PSUM"))
        psr = ctx.enter_context(tc.tile_pool(name="psr", bufs=2, space="PSUM"))
        s16p = ctx.enter_context(tc.tile_pool(name="s16", bufs=2))
        e16p = ctx.enter_context(tc.tile_pool(name="e16", bufs=2))
        ebp = ctx.enter_context(tc.tile_pool(name="eb", bufs=2))
        abp = ctx.enter_context(tc.tile_pool(name="ab", bufs=2))
        xgp = ctx.enter_context(tc.tile_pool(name="xg", bufs=2))
        sm = ctx.enter_context(tc.tile_pool(name="sm", bufs=4))

        ablk_s = const.tile([P, Q * H], f16)
        nc.sync.dma_start(ablk_s[:], ablk_d[:, :])
        aar_s = const.tile([F, H], f32)
        nc.sync.dma_start(aar_s[:], aar_d[:, :])
        npad_s = const.tile([P, J], f32)
        nc.sync.dma_start(npad_s[:], npad_d[:, :])

        # ---- s_row for own nodes + pad-correction factors ----
        xto_s = const.tile([F, J * P], f32)
        nc.sync.dma_start(xto_s[:], xto_d[:, :])
        srow = const.tile([P, J * H], f32)
        padex = const.tile([P, J * H], f32)
        for j in range(J):
            if Cj[j] == 0:
                continue
            ps = psr.tile([P, H], f32)
            nc.tensor.matmul(ps[:], lhsT=xto_s[:, j * P:(j + 1) * P],
                             rhs=aar_s[:], start=True, stop=True)
            nc.vector.tensor_copy(srow[:, j * H:(j + 1) * H], ps[:])
        # padex = exp(lrelu(srow))
        nc.vector.scalar_tensor_tensor(
            out=padex[:], in0=srow[:], scalar=ALPHA, in1=srow[:],
            op0=mybir.AluOpType.mult, op1=mybir.AluOpType.max)
        nc.scalar.activation(padex[:], padex[:],
                             mybir.ActivationFunctionType.Exp)

        # ---- batched pipeline over c-groups ----
        for bi, (j0, j1) in enumerate(batches):
            cb0, cb1 = C0[j0], C0[j1]
            W = cb1 - cb0                 # c-groups in batch
            if W == 0:
                continue
            cols = W * P

            xg = xgp.tile([P, BATCH_C * P], f16, tag="xg")
            nc.sync.dma_start(xg[:, :cols], xg_d[:, cb0 * P:cb1 * P])

            s16 = s16p.tile([32, BATCH_C * P], f16, tag="s16")
            n_mm = -(-cols // MM_N)
            for m in range(n_mm):
                lo = m * MM_N
                hi = min(cols, lo + MM_N)
                ps = psc.tile([32, MM_N], f32, tag="psmm")
                nc.tensor.matmul(ps[:, :hi - lo], lhsT=ablk_s[:],
                                 rhs=xg[:, lo:hi], start=True, stop=True)
                eng = nc.vector if m % 2 == 0 else nc.scalar
                if eng is nc.vector:
                    eng.tensor_copy(s16[:, lo:hi], ps[:, :hi - lo])
                else:
                    eng.activation(s16[:, lo:hi], ps[:, :hi - lo],
                                   mybir.ActivationFunctionType.Copy)

            # [32, W*128] -> [128, 32*W]; out[p, r*W + c] = s16[r, c*128+p]
            blk = s16d[bi, :32 * cols]
            nc.sync.dma_start(blk.rearrange("(r x) -> r x", r=32),
                              s16[:, :cols])
            e16 = e16p.tile([P, 32 * BATCH_C], f16, tag="e16")
            nc.sync.dma_start(e16[:, :32 * W],
                              blk.rearrange("(m p) -> m p", p=P),
                              transpose=True)
            if debug and bi == 0:
                nc.sync.dma_start(dbg_s16[:, :cols], s16[:, :cols])
                nc.sync.dma_start(dbg_e16[:, :32 * W], e16[:, :32 * W])
                nc.sync.dma_start(dbg_srow[:, :], srow[:])

            eb = ebp.tile([P, 32 * BATCH_C], f32, tag="eb")
            ab = abp.tile([P, 32 * BATCH_C], f32, tag="ab")

            # add s_row (broadcast over q and c) per tile
            for j in range(j0, j1):
                lc = C0[j] - cb0
                Cw = Cj[j]
                if Cw == 0:
                    continue
                e3 = _v3(eb, W, lc, Cw)
                g3 = _v3(e16, W, lc, Cw)
                srj = (srow[:, j * H:(j + 1) * H]
                       .unsqueeze(1).unsqueeze(3)
                       .broadcast_to([P, Q, H, Cw]))
                nc.vector.tensor_tensor(out=e3, in0=g3, in1=srj,
                                        op=mybir.AluOpType.add)
            # lrelu + exp over the whole contiguous batch buffer
            flat = eb[:, :32 * W]
            nc.vector.scalar_tensor_tensor(
                out=flat, in0=flat, scalar=ALPHA, in1=flat,
                op0=mybir.AluOpType.mult, op1=mybir.AluOpType.max)
            nc.scalar.activation(flat, flat,
                                 mybir.ActivationFunctionType.Exp)
            if debug and bi == 0:
                nc.sync.dma_start(dbg_eb[:, :32 * W], eb[:, :32 * W])

            for j in range(j0, j1):
                lc = C0[j] - cb0
                Cw = Cj[j]
                if Cw == 0:
                    continue
                e3 = _v3(eb, W, lc, Cw)
                # denominator: sum over c then q; subtract pad contribution
                s32 = sm.tile([P, 32], f32, tag="s32")
                nc.vector.tensor_reduce(
                    out=s32[:].unsqueeze(2), in_=e3,
                    axis=mybir.AxisListType.X, op=mybir.AluOpType.add)
                s8 = sm.tile([P, H], f32, tag="s8")
                nc.vector.tensor_reduce(
                    out=s8[:].unsqueeze(1),
                    in_=s32[:].rearrange("p (q h) -> p q h", h=H)
                             .transpose([0, 2, 1]),
                    axis=mybir.AxisListType.X, op=mybir.AluOpType.add)
                pj = sm.tile([P, H], f32, tag="pj")
                nc.vector.tensor_scalar(
                    out=pj[:], in0=padex[:, j * H:(j + 1) * H],
                    scalar1=npad_s[:, j:j + 1], scalar2=None,
                    op0=mybir.AluOpType.mult)
                nc.vector.tensor_sub(s8[:], s8[:], pj[:])
                nc.vector.tensor_scalar_add(s8[:], s8[:], EPS)
                nc.vector.reciprocal(s8[:], s8[:])
                # numerators * recip, repacked to d-major [c, q, h]
                av = (ab[:, lc * 32:(lc + Cw) * 32]
                      .rearrange("p (c q h) -> p c q h", q=Q, h=H))
                ev = e3.transpose([0, 3, 1, 2])       # [P, Cw, Q, H]
                rv = (s8[:].unsqueeze(1).unsqueeze(2)
                      .broadcast_to([P, Cw, Q, H]))
                nc.vector.tensor_tensor(out=av, in0=ev, in1=rv,
                                        op=mybir.AluOpType.mult)
            nc.sync.dma_start(out_d[:, cb0 * 32:cb1 * 32], ab[:, :32 * W])

    nc.compile()
    return nc


def _v3(buf, W, lc, Cw):
    """[P, q, h, c] view of a tile-j slice inside a batch buffer."""
    return (buf[:, :32 * W]
            .rearrange("p (q h c) -> p q h c", q=Q, h=H)[:, :, :, lc:lc + Cw])


def _get_program(key_args):
    key = tuple(sorted((k, v) for k, v in key_args.items()))
    if key not in _prog_cache:
        _prog_cache[key] = _build_program(**key_args)
    return _prog_cache[key]


# --------------------------------------------------------------------------
# entry point
# --------------------------------------------------------------------------

def kernel(x, aa, row, col):
    inputs, meta = _host_prep(x, aa, row, col)

    from concourse.bass_utils import run_bass_kernel_spmd

    nc = _get_program(dict(J=meta["J"], C_tot=meta["C_tot"],
                           ncores=meta["ncores"], Cj=meta["Cj"],
                           batches=meta["batches"]))

    in_maps = []
    for c in range(meta["ncores"]):
        in_maps.append({
            "xg4": inputs["xg4"][c],
            "xto": inputs["xto"][c],
            "aar": inputs["aarT"],
            "ablk": inputs["aablk"],
            "npad": inputs["npad"][c],
        })
    res = run_bass_kernel_spmd(nc, in_maps,
                               core_ids=list(range(meta["ncores"])))
    global LAST_RESULT
    LAST_RESULT = res
    outs = [res.results[c]["out"].reshape(-1) for c in range(meta["ncores"])]
    return _unshard(outs, meta)


def _unshard(outs, meta):
    E = meta["E"]
    a = np.empty((H, E), np.float32)
    sidx = meta["sidx"]
    c_e = meta["c_e"]
    out_flat = meta["out_flat"]
    for c in range(meta["ncores"]):
        m = c_e == c
        base = out_flat[m]
        dst = sidx[m]
        src = outs[c]
        for h in range(H):
            a[h, dst] = src[base + h]
    return a

